# revision 33
# baseline (speedup 1.0000x reference)
"""Multi-head attention (B=2, N=2048, D=1024, H=16, hd=64) on 8 trn2 NeuronCores.

Sharding: 8 cores = 2 (batch) x 4 (head groups of 4 heads).
Core c: batch b = c // 4, heads hg*4 .. hg*4+3 where hg = c % 4.

Per-core program (identical SPMD program, per-core data):
  inputs (DRAM):
    xT     [1024, 2048]  = x[b].T
    wqkT   [1024, 512]   = w_qkv[[q rows, k rows] of local heads].T
    wvT    [1024, 256]   = w_qkv[v rows of local heads].T
    wprojT [256, 1024]   = w_proj[:, local head cols].T
  output:
    out    [2048, 1024]  partial (row-parallel) projection output

Pipeline (v2, ACT-exp is the rail at ~146us):
  - During the x DMA (12.6us floor), accumulate k-gemm (m2) and half the
    v-gemm kt-outer so the PE works while x streams in.
  - Attention chains per head-PAIR with row-tiled score matmuls: head even
    at array rows 0-63 (tile_position (0,0)), head odd at rows 64-127
    ((64,0)) -> both score MMs run concurrently; one [128,1024] exp per
    key tile covers both heads.
  - PV per head ones-augmented (65th weight column = denominator row).
  - Chain order: hp0 for qb0..3, then hp1 for qb0..3, so the hp1 q/k gemms
    and the projections become PE filler inside the ACT-bound stream.
  - Normalize: reciprocal_approx_fast (DVE) on the denominator row, f32r
    outer-product broadcast on the PE (interleaved into the next chain),
    DVE multiply, DMA into proj layout.

Host unshard: out[b] = sum over 4 head-group partials + b_proj.
"""

import sys

if "/opt/trn_rl_repo" not in sys.path:
    sys.path.insert(0, "/opt/trn_rl_repo")

import numpy as np

B, N, D, H, HD = 2, 2048, 1024, 16, 64
NCORES = 8
HPC = 4               # heads per core
LQK = HPC * HD        # 256 local q (or k) rows
SCALE = HD ** -0.5    # 0.125

_CACHE = {}


def _build_program():
    import concourse.tile as tile
    from concourse import bacc, mybir

    F32 = mybir.dt.float32
    BF16 = mybir.dt.bfloat16
    Exp = mybir.ActivationFunctionType.Exp

    nc = bacc.Bacc("TRN2", target_bir_lowering=False, debug=False,
                   num_devices=NCORES)

    xT_d = nc.dram_tensor("xT", [D, N], BF16, kind="ExternalInput").ap()
    wqkT_d = nc.dram_tensor("wqkT", [D, 2 * LQK], BF16, kind="ExternalInput").ap()
    wvT_d = nc.dram_tensor("wvT", [D, LQK], BF16, kind="ExternalInput").ap()
    wprojT_d = nc.dram_tensor("wprojT", [LQK, D], BF16, kind="ExternalInput").ap()
    out_d = nc.dram_tensor("out", [N, D], F32, kind="ExternalOutput").ap()

    KT = D // 128        # 8 contraction tiles for qkv gemms
    NB = N // 512        # 4 seq blocks
    NT = N // 128        # 16 seq tiles

    with tile.TileContext(nc) as tc:
        with (
            nc.allow_low_precision(reason="bf16 matmul operands"),
            tc.tile_pool(name="const", bufs=1) as cpool,
            tc.tile_pool(name="w", bufs=1) as wpool,
            tc.tile_pool(name="x", bufs=1) as xpool,
            tc.tile_pool(name="qk", bufs=1) as qkpool,
            tc.tile_pool(name="vaug", bufs=1) as vapool,
            tc.tile_pool(name="ao", bufs=1) as aopool,
            tc.tile_pool(name="probs", bufs=3) as prpool,
            tc.tile_pool(name="small", bufs=4) as smpool,
            tc.tile_pool(name="stage", bufs=3) as stpool,
            tc.tile_pool(name="scp", bufs=2, space="PSUM") as scp,   # 2x[128,1024]
            tc.tile_pool(name="pvp", bufs=2, space="PSUM") as pvp,   # 2x[128,512]
            tc.tile_pool(name="pjp", bufs=2, space="PSUM") as pjp,   # 2x[128,512]
        ):
            ones_f32 = cpool.tile([128, 128], F32)
            nc.vector.memset(ones_f32[:, :], 1.0)
            ones_bf = cpool.tile([65, 128], BF16)
            nc.vector.tensor_copy(ones_bf[:, :], ones_f32[0:65, :])

            # ---- input DMAs (kt-chunked so compute starts early) ----
            x_sb = xpool.tile([128, KT, N], BF16)
            wqk_sb = wpool.tile([128, KT, 2 * LQK], BF16)
            wv_sb = wpool.tile([128, KT, LQK], BF16)
            xT_r = xT_d.rearrange("(kt p) n -> p kt n", p=128)
            wqkT_r = wqkT_d.rearrange("(kt p) m -> p kt m", p=128)
            wvT_r = wvT_d.rearrange("(kt p) m -> p kt m", p=128)
            # One queue processes transfers in order, so: whole wqk first
            # (kq gemm weights), first x chunk, wv, then remaining x chunks
            # kt-granular so the kt-outer gemm tracks arrivals; wproj last.
            nc.sync.dma_start(out=wqk_sb[:, :, :], in_=wqkT_r[:, :, :])
            nc.sync.dma_start(out=x_sb[:, 0, :], in_=xT_r[:, 0, :])
            nc.sync.dma_start(out=wv_sb[:, :, :], in_=wvT_r[:, :, :])
            for kt in range(1, KT):
                nc.sync.dma_start(out=x_sb[:, kt, :], in_=xT_r[:, kt, :])
            wproj_sb = wpool.tile([128, 2, D], BF16)
            nc.sync.dma_start(
                out=wproj_sb[:, :, :],
                in_=wprojT_d.rearrange("(kt p) o -> p kt o", p=128))

            # qk_sb m-tile layout: m=0: q heads 0,1 / m=1: q heads 2,3
            #                      m=2: k heads 0,1 / m=3: k heads 2,3
            qk_sb = qkpool.tile([128, 4, N], BF16)
            v_sb = vapool.tile([128, NT, HPC, HD + 1], BF16)
            # proj lhsT: kt2=hp, partitions: even head 0:64, odd head 64:128
            ao_sb = aopool.tile([128, 2, N], BF16)

            WCOL = (0, 128, 256, 384)  # wqkT col base per m-tile

            def v_copy(ps, st0, cnt):
                """Copy cnt [128, LQK] psum v-slices (st0..) into v_sb."""
                for j in range(cnt):
                    nc.vector.tensor_copy(
                        v_sb[:, st0 + j, :, 0:HD],
                        ps[:, j * LQK:(j + 1) * LQK].rearrange(
                            "p (h d) -> p h d", h=HPC))
                    nc.vector.tensor_copy(
                        v_sb[:, st0 + j, :, HD:HD + 1],
                        ones_f32[:, 0:HPC].rearrange("p (h c) -> p h c", c=1))

            # ---- gemm phase overlapped with the x DMA (kt-outer) ----
            # NOTE: interleaved accumulation groups must not share a PSUM
            # bank (start=True clears bank-wide state), so each concurrent
            # accumulation gets its own bank: m2 nb0-3 (2 banks each of the
            # two scp tiles), v st0/st1/st2 (one bank each), m0 nb0.
            m2ps_a = scp.tile([128, 1024], F32, tag="sc")
            m2ps_b = scp.tile([128, 1024], F32, tag="sc")
            vps0 = pvp.tile([128, 512], F32, tag="pv")
            vps1 = pvp.tile([128, 512], F32, tag="pv")
            vps2 = pjp.tile([128, 512], F32, tag="pj")
            m0ps = pjp.tile([128, 512], F32, tag="pj")
            for kt in range(KT):
                st_, sp_ = (kt == 0), (kt == KT - 1)
                for nb in range(NB):
                    ps = m2ps_a if nb < 2 else m2ps_b
                    nc.tensor.matmul(
                        ps[:, (nb % 2) * 512:(nb % 2 + 1) * 512],
                        wqk_sb[:, kt, WCOL[2]:WCOL[2] + 128],
                        x_sb[:, kt, nb * 512:(nb + 1) * 512],
                        start=st_, stop=sp_)
                nc.tensor.matmul(
                    m0ps[:, :],
                    wqk_sb[:, kt, WCOL[0]:WCOL[0] + 128],
                    x_sb[:, kt, 0:512],
                    start=st_, stop=sp_)
                for st in range(3):
                    ps = (vps0, vps1, vps2)[st]
                    nc.tensor.matmul(
                        ps[:, 0:LQK],
                        x_sb[:, kt, st * 128:(st + 1) * 128],
                        wv_sb[:, kt, :],
                        start=st_, stop=sp_)
            # copies ordered so C0's first tiles unblock first
            nc.vector.tensor_copy(qk_sb[:, 0, 0:512], m0ps[:, :])
            nc.vector.tensor_copy(qk_sb[:, 2, 0:512], m2ps_a[:, 0:512])
            v_copy(vps0, 0, 1)
            nc.vector.tensor_copy(qk_sb[:, 2, 512:1024], m2ps_a[:, 512:1024])
            v_copy(vps1, 1, 1)
            v_copy(vps2, 2, 1)
            nc.vector.tensor_copy(qk_sb[:, 2, 1024:1536], m2ps_b[:, 0:512])
            nc.vector.tensor_copy(qk_sb[:, 2, 1536:2048], m2ps_b[:, 512:1024])

            # ---- filler generators (kt-inner gemm pieces fed into chains) --

            def qk_fill(m, nb):
                def emit():
                    ps = pjp.tile([128, 512], F32, tag="pj", name=f"qkf{m}{nb}")
                    for kt in range(KT):
                        nc.tensor.matmul(
                            ps[:, :],
                            wqk_sb[:, kt, WCOL[m]:WCOL[m] + 128],
                            x_sb[:, kt, nb * 512:(nb + 1) * 512],
                            start=(kt == 0), stop=(kt == KT - 1))
                    nc.vector.tensor_copy(
                        qk_sb[:, m, nb * 512:(nb + 1) * 512], ps[:, :])
                return emit

            def v_fill(st0):
                def emit():
                    ps = pjp.tile([128, 512], F32, tag="pj", name=f"vf{st0}")
                    for kt in range(KT):
                        nc.tensor.matmul(
                            ps[:, 0:LQK],
                            x_sb[:, kt, st0 * 128:(st0 + 1) * 128],
                            wv_sb[:, kt, :],
                            start=(kt == 0), stop=(kt == KT - 1))
                    v_copy(ps, st0, 1)
                return emit

            def proj_fill(qb, j):
                nt = qb * 4 + j

                def emit():
                    outst = stpool.tile([128, 1024], F32, tag="outst",
                                        name=f"outst{qb}{nt}")
                    for ob in range(2):
                        ps = pjp.tile([128, 512], F32, tag="pj",
                                      name=f"pjps{qb}{nt}{ob}")
                        for kt2 in range(2):
                            nc.tensor.matmul(
                                ps[:, :],
                                ao_sb[:, kt2, nt * 128:(nt + 1) * 128],
                                wproj_sb[:, kt2, ob * 512:(ob + 1) * 512],
                                start=(kt2 == 0), stop=(kt2 == 1))
                        nc.vector.tensor_copy(
                            outst[:, ob * 512:(ob + 1) * 512], ps[:, :])
                    nc.sync.dma_start(
                        out=out_d[nt * 128:(nt + 1) * 128, :], in_=outst[:, :])
                return emit

            # ---- normalize tail, split so the PE bc matmul lands inside the
            # next chain (recip has time to finish on the DVE) ----

            def norm_pvs(hp, qb, pv_e, pv_o):
                """Copy the pv accumulators to SBUF: frees their PSUM banks
                quickly. Reciprocals/normalize run 1-2 chains later."""
                pvs = stpool.tile([65, 1024], F32, tag="pvs",
                                  name=f"pvs{hp}{qb}")
                nc.vector.tensor_copy(pvs[0:65, 0:512], pv_e[0:65, :])
                nc.vector.tensor_copy(pvs[0:65, 512:1024], pv_o[0:65, :])
                return pvs

            def norm_recips(e):
                rcr = smpool.tile([65, 1024], BF16, tag="rcr",
                                  name=f"rcr{e['hp']}{e['qb']}")
                nc.vector.reciprocal(rcr[64:65, 0:512], e['pvs'][64:65, 0:512])
                nc.vector.reciprocal(rcr[64:65, 512:1024],
                                     e['pvs'][64:65, 512:1024])
                return rcr

            def norm_apply(e):
                """bc matmul (PE, bf16) + DVE multiply + DMA into ao_sb."""
                hp, qb, parity = e['hp'], e['qb'], e['applied']
                pvs, rcr = e['pvs'], e['rcr']
                pi = parity * 64
                bc = pjp.tile([64, 512], F32, tag="pj", name=f"bc{hp}{qb}{parity}")
                nc.tensor.matmul(
                    bc[:, :], ones_bf[64:65, 0:64],
                    rcr[64:65, parity * 512:(parity + 1) * 512],
                    start=True, stop=True)
                aos = stpool.tile([64, 512], BF16, tag="aos",
                                  name=f"aos{hp}{qb}{parity}")
                nc.vector.tensor_mul(
                    aos[:, :], bc[:, :],
                    pvs[0:64, parity * 512:(parity + 1) * 512])
                nc.sync.dma_start(
                    out=ao_sb[pi:pi + 64, hp, qb * 512:(qb + 1) * 512],
                    in_=aos[:, :])
                e['applied'] += 1

            # ---- attention chain per head pair, with filler injection ----
            pending = []   # [{ci, hp, qb, pvs, rcr, applied}, ...]
            chain_no = [0]

            def chain(hp, qb, fillers):
                """fillers: dict slot -> list of emit closures."""
                ci = chain_no[0]
                chain_no[0] += 1
                mq, mk = hp, 2 + hp
                qT_e = qk_sb[0:64, mq, qb * 512:(qb + 1) * 512]
                qT_o = qk_sb[64:128, mq, qb * 512:(qb + 1) * 512]
                pv_e = pvp.tile([128, 512], F32, tag="pv", name=f"pve{hp}{qb}")
                pv_o = pvp.tile([128, 512], F32, tag="pv", name=f"pvo{hp}{qb}")
                for kt in range(NT):
                    sc = scp.tile([128, 1024], F32, tag="sc", name=f"sc{kt}")
                    nc.tensor.matmul(
                        sc[:, 0:512],
                        qk_sb[0:64, mk, kt * 128:(kt + 1) * 128], qT_e,
                        start=True, stop=True)
                    nc.tensor.matmul(
                        sc[:, 512:1024],
                        qk_sb[64:128, mk, kt * 128:(kt + 1) * 128], qT_o,
                        start=True, stop=True)
                    pr = prpool.tile([128, 1024], BF16, tag="probs",
                                     name=f"pr{kt}")
                    nc.scalar.activation(pr[:, :], sc[:, :], Exp, scale=SCALE)
                    # normalize deferred by two chains; reciprocals emitted
                    # mid-previous-chain so neither ever blocks the PE or
                    # delays norm DVE work through the DVE FIFO
                    if kt in (2, 5):
                        for e in pending:
                            if e['ci'] <= ci - 2 and e['applied'] < 2:
                                norm_apply(e)
                                break
                    if kt == 7:
                        for e in pending:
                            if e['ci'] == ci - 1 and e['rcr'] is None:
                                e['rcr'] = norm_recips(e)
                    for emit in fillers.get(kt, ()):
                        emit()
                    nc.tensor.matmul(
                        pv_e[0:65, :], v_sb[:, kt, 2 * hp, 0:HD + 1],
                        pr[:, 0:512],
                        start=(kt == 0), stop=(kt == NT - 1))
                    nc.tensor.matmul(
                        pv_o[0:65, :], v_sb[:, kt, 2 * hp + 1, 0:HD + 1],
                        pr[:, 512:1024],
                        start=(kt == 0), stop=(kt == NT - 1))
                pvs = norm_pvs(hp, qb, pv_e, pv_o)
                pending.append({'ci': ci, 'hp': hp, 'qb': qb, 'pvs': pvs,
                                'rcr': None, 'applied': 0})
                while pending and pending[0]['applied'] >= 2:
                    pending.pop(0)

            c0_fills = {s: [v_fill(3 + s)] for s in range(13)}
            c0_fills[13] = [qk_fill(0, 1)]
            chain(0, 0, c0_fills)
            chain(0, 1, {1: [qk_fill(3, 0)], 4: [qk_fill(3, 1)],
                         8: [qk_fill(3, 2)], 12: [qk_fill(0, 2)]})
            chain(0, 2, {1: [qk_fill(3, 3)], 4: [qk_fill(1, 0)],
                         8: [qk_fill(1, 1)], 12: [qk_fill(0, 3)]})
            chain(0, 3, {1: [qk_fill(1, 2)], 4: [qk_fill(1, 3)]})
            chain(1, 0, {})
            chain(1, 1, {})
            chain(1, 2, {7: [proj_fill(0, 0)], 9: [proj_fill(0, 1)],
                         11: [proj_fill(0, 2)], 13: [proj_fill(0, 3)]})
            chain(1, 3, {7: [proj_fill(1, 0)], 9: [proj_fill(1, 1)],
                         11: [proj_fill(1, 2)], 13: [proj_fill(1, 3)]})
            # tail: C7's recips first (they run on the DVE while the PE does
            # C6's norms and proj(2)), then the remaining norms and projs
            for e in pending:
                if e['rcr'] is None:
                    e['rcr'] = norm_recips(e)
            live = [e for e in pending if e['applied'] < 2]
            norm_apply(live[0])
            norm_apply(live[0])
            for j in range(4):
                proj_fill(2, j)()
            norm_apply(live[1])
            norm_apply(live[1])
            for j in range(4):
                proj_fill(3, j)()

    nc.compile()
    return nc


def _get_program():
    if "nc" not in _CACHE:
        _CACHE["nc"] = _build_program()
    return _CACHE["nc"]


def _make_in_maps(x, w_qkv, w_proj):
    import ml_dtypes
    bf16 = ml_dtypes.bfloat16
    x = np.asarray(x, dtype=np.float32)
    w_qkv = np.asarray(w_qkv, dtype=np.float32)
    w_proj = np.asarray(w_proj, dtype=np.float32)
    xT = [np.ascontiguousarray(x[b].T).astype(bf16) for b in range(B)]
    in_maps = []
    for c in range(NCORES):
        b, hg = c // 4, c % 4
        rows = slice(hg * LQK, (hg + 1) * LQK)
        qk_rows = np.r_[np.arange(hg * LQK, (hg + 1) * LQK),
                        D + np.arange(hg * LQK, (hg + 1) * LQK)]
        in_maps.append({
            "xT": xT[b],
            "wqkT": np.ascontiguousarray(w_qkv[qk_rows, :].T).astype(bf16),
            "wvT": np.ascontiguousarray(
                w_qkv[2 * D + np.arange(hg * LQK, (hg + 1) * LQK), :].T).astype(bf16),
            "wprojT": np.ascontiguousarray(w_proj[:, rows].T).astype(bf16),
        })
    return in_maps


def kernel(x, w_qkv, w_proj, b_proj, _return_results=False, _trace=False):
    from concourse import bass_utils

    nc = _get_program()
    in_maps = _make_in_maps(x, w_qkv, w_proj)
    res = bass_utils.run_bass_kernel_spmd(
        nc, in_maps, list(range(NCORES)), trace=_trace)
    partials = np.stack([res.results[c]["out"] for c in range(NCORES)])
    out = partials.reshape(B, 4, N, D).sum(axis=1, dtype=np.float32)
    out = out + np.asarray(b_proj, dtype=np.float32)[None, None, :]
    out = out.astype(np.float32)
    if _return_results:
        return out, res
    return out


# revision 35
# speedup vs baseline: 1.0270x; 1.0270x over previous
"""Multi-head attention (B=2, N=2048, D=1024, H=16, hd=64) on 8 trn2 NeuronCores.

Sharding: 8 cores = 2 (batch) x 4 (head groups of 4 heads).
Core c: batch b = c // 4, heads hg*4 .. hg*4+3 where hg = c % 4.

Per-core program (identical SPMD program, per-core data):
  inputs (DRAM):
    xT     [1024, 2048]  = x[b].T
    wqkT   [1024, 512]   = w_qkv[[q rows, k rows] of local heads].T
    wvT    [1024, 256]   = w_qkv[v rows of local heads].T
    wprojT [256, 1024]   = w_proj[:, local head cols].T
  output:
    out    [2048, 1024]  partial (row-parallel) projection output

Pipeline (v2, ACT-exp is the rail at ~146us):
  - During the x DMA (12.6us floor), accumulate k-gemm (m2) and half the
    v-gemm kt-outer so the PE works while x streams in.
  - Attention chains per head-PAIR with row-tiled score matmuls: head even
    at array rows 0-63 (tile_position (0,0)), head odd at rows 64-127
    ((64,0)) -> both score MMs run concurrently; one [128,1024] exp per
    key tile covers both heads.
  - PV per head ones-augmented (65th weight column = denominator row).
  - Chain order: hp0 for qb0..3, then hp1 for qb0..3, so the hp1 q/k gemms
    and the projections become PE filler inside the ACT-bound stream.
  - Normalize: reciprocal_approx_fast (DVE) on the denominator row, f32r
    outer-product broadcast on the PE (interleaved into the next chain),
    DVE multiply, DMA into proj layout.

Host unshard: out[b] = sum over 4 head-group partials + b_proj.
"""

import sys

if "/opt/trn_rl_repo" not in sys.path:
    sys.path.insert(0, "/opt/trn_rl_repo")

import numpy as np

B, N, D, H, HD = 2, 2048, 1024, 16, 64
NCORES = 8
HPC = 4               # heads per core
LQK = HPC * HD        # 256 local q (or k) rows
SCALE = HD ** -0.5    # 0.125

_CACHE = {}


def _build_program():
    import concourse.tile as tile
    from concourse import bacc, mybir

    F32 = mybir.dt.float32
    BF16 = mybir.dt.bfloat16
    Exp = mybir.ActivationFunctionType.Exp

    nc = bacc.Bacc("TRN2", target_bir_lowering=False, debug=False,
                   num_devices=NCORES)

    xT_d = nc.dram_tensor("xT", [D, N], BF16, kind="ExternalInput").ap()
    wqkT_d = nc.dram_tensor("wqkT", [D, 2 * LQK], BF16, kind="ExternalInput").ap()
    wvT_d = nc.dram_tensor("wvT", [D, LQK], BF16, kind="ExternalInput").ap()
    wprojT_d = nc.dram_tensor("wprojT", [LQK, D], BF16, kind="ExternalInput").ap()
    out_d = nc.dram_tensor("out", [N, D], F32, kind="ExternalOutput").ap()

    KT = D // 128        # 8 contraction tiles for qkv gemms
    NB = N // 512        # 4 seq blocks
    NT = N // 128        # 16 seq tiles

    with tile.TileContext(nc) as tc:
        with (
            nc.allow_low_precision(reason="bf16 matmul operands"),
            tc.tile_pool(name="const", bufs=1) as cpool,
            tc.tile_pool(name="w", bufs=1) as wpool,
            tc.tile_pool(name="x", bufs=1) as xpool,
            tc.tile_pool(name="qk", bufs=1) as qkpool,
            tc.tile_pool(name="vaug", bufs=1) as vapool,
            tc.tile_pool(name="ao", bufs=1) as aopool,
            tc.tile_pool(name="probs", bufs=3) as prpool,
            tc.tile_pool(name="small", bufs=4) as smpool,
            tc.tile_pool(name="stage", bufs=3) as stpool,
            tc.tile_pool(name="scp", bufs=2, space="PSUM") as scp,   # 2x[128,1024]
            tc.tile_pool(name="pvp", bufs=2, space="PSUM") as pvp,   # 2x[128,512]
            tc.tile_pool(name="pjp", bufs=2, space="PSUM") as pjp,   # 2x[128,512]
        ):
            ones_f32 = cpool.tile([128, 128], F32)
            nc.vector.memset(ones_f32[:, :], 1.0)
            ones_bf = cpool.tile([65, 128], BF16)
            nc.vector.tensor_copy(ones_bf[:, :], ones_f32[0:65, :])

            # ---- input DMAs (kt-chunked so compute starts early) ----
            x_sb = xpool.tile([128, KT, N], BF16)
            wqk_sb = wpool.tile([128, KT, 2 * LQK], BF16)
            wv_sb = wpool.tile([128, KT, LQK], BF16)
            xT_r = xT_d.rearrange("(kt p) n -> p kt n", p=128)
            wqkT_r = wqkT_d.rearrange("(kt p) m -> p kt m", p=128)
            wvT_r = wvT_d.rearrange("(kt p) m -> p kt m", p=128)
            # One queue processes transfers in order, so: whole wqk first
            # (kq gemm weights), first x chunk, wv, then remaining x chunks
            # kt-granular so the kt-outer gemm tracks arrivals; wproj last.
            nc.sync.dma_start(out=wqk_sb[:, :, :], in_=wqkT_r[:, :, :])
            nc.sync.dma_start(out=x_sb[:, 0, :], in_=xT_r[:, 0, :])
            nc.sync.dma_start(out=wv_sb[:, :, :], in_=wvT_r[:, :, :])
            for kt in range(1, KT):
                nc.sync.dma_start(out=x_sb[:, kt, :], in_=xT_r[:, kt, :])
            wproj_sb = wpool.tile([128, 2, D], BF16)
            nc.sync.dma_start(
                out=wproj_sb[:, :, :],
                in_=wprojT_d.rearrange("(kt p) o -> p kt o", p=128))

            # qk_sb m-tile layout: m=0: q heads 0,1 / m=1: q heads 2,3
            #                      m=2: k heads 0,1 / m=3: k heads 2,3
            qk_sb = qkpool.tile([128, 4, N], BF16)
            v_sb = vapool.tile([128, NT, HPC, HD + 1], BF16)
            # proj lhsT: kt2=hp, partitions: even head 0:64, odd head 64:128
            ao_sb = aopool.tile([128, 2, N], BF16)

            WCOL = (0, 128, 256, 384)  # wqkT col base per m-tile

            def v_copy(ps, st0, cnt):
                """Copy cnt [128, LQK] psum v-slices (st0..) into v_sb."""
                for j in range(cnt):
                    nc.vector.tensor_copy(
                        v_sb[:, st0 + j, :, 0:HD],
                        ps[:, j * LQK:(j + 1) * LQK].rearrange(
                            "p (h d) -> p h d", h=HPC))
                    nc.vector.tensor_copy(
                        v_sb[:, st0 + j, :, HD:HD + 1],
                        ones_f32[:, 0:HPC].rearrange("p (h c) -> p h c", c=1))

            # ---- gemm phase overlapped with the x DMA (kt-outer) ----
            # NOTE: interleaved accumulation groups must not share a PSUM
            # bank (start=True clears bank-wide state), so each concurrent
            # accumulation gets its own bank: m2 nb0-3 (2 banks each of the
            # two scp tiles), v st0/st1/st2 (one bank each), m0 nb0.
            m2ps_a = scp.tile([128, 1024], F32, tag="sc")
            m2ps_b = scp.tile([128, 1024], F32, tag="sc")
            vps0 = pvp.tile([128, 512], F32, tag="pv")
            vps1 = pvp.tile([128, 512], F32, tag="pv")
            vps2 = pjp.tile([128, 512], F32, tag="pj")
            m0ps = pjp.tile([128, 512], F32, tag="pj")
            for kt in range(KT):
                st_, sp_ = (kt == 0), (kt == KT - 1)
                for nb in range(NB):
                    ps = m2ps_a if nb < 2 else m2ps_b
                    nc.tensor.matmul(
                        ps[:, (nb % 2) * 512:(nb % 2 + 1) * 512],
                        wqk_sb[:, kt, WCOL[2]:WCOL[2] + 128],
                        x_sb[:, kt, nb * 512:(nb + 1) * 512],
                        start=st_, stop=sp_)
                nc.tensor.matmul(
                    m0ps[:, :],
                    wqk_sb[:, kt, WCOL[0]:WCOL[0] + 128],
                    x_sb[:, kt, 0:512],
                    start=st_, stop=sp_)
                for st in range(3):
                    ps = (vps0, vps1, vps2)[st]
                    nc.tensor.matmul(
                        ps[:, 0:LQK],
                        x_sb[:, kt, st * 128:(st + 1) * 128],
                        wv_sb[:, kt, :],
                        start=st_, stop=sp_)
            # copies ordered so C0's first tiles unblock first
            nc.vector.tensor_copy(qk_sb[:, 0, 0:512], m0ps[:, :])
            nc.vector.tensor_copy(qk_sb[:, 2, 0:512], m2ps_a[:, 0:512])
            v_copy(vps0, 0, 1)
            nc.vector.tensor_copy(qk_sb[:, 2, 512:1024], m2ps_a[:, 512:1024])
            v_copy(vps1, 1, 1)
            v_copy(vps2, 2, 1)
            nc.vector.tensor_copy(qk_sb[:, 2, 1024:1536], m2ps_b[:, 0:512])
            nc.vector.tensor_copy(qk_sb[:, 2, 1536:2048], m2ps_b[:, 512:1024])

            # ---- filler generators (kt-inner gemm pieces fed into chains) --

            def qk_fill(m, nb):
                def emit():
                    ps = pjp.tile([128, 512], F32, tag="pj", name=f"qkf{m}{nb}")
                    for kt in range(KT):
                        nc.tensor.matmul(
                            ps[:, :],
                            wqk_sb[:, kt, WCOL[m]:WCOL[m] + 128],
                            x_sb[:, kt, nb * 512:(nb + 1) * 512],
                            start=(kt == 0), stop=(kt == KT - 1))
                    nc.vector.tensor_copy(
                        qk_sb[:, m, nb * 512:(nb + 1) * 512], ps[:, :])
                return emit

            def v_fill(st0):
                def emit():
                    ps = pjp.tile([128, 512], F32, tag="pj", name=f"vf{st0}")
                    for kt in range(KT):
                        nc.tensor.matmul(
                            ps[:, 0:LQK],
                            x_sb[:, kt, st0 * 128:(st0 + 1) * 128],
                            wv_sb[:, kt, :],
                            start=(kt == 0), stop=(kt == KT - 1))
                    v_copy(ps, st0, 1)
                return emit

            def proj_fill(qb, j):
                nt = qb * 4 + j

                def emit():
                    outst = stpool.tile([128, 1024], F32, tag="outst",
                                        name=f"outst{qb}{nt}")
                    for ob in range(2):
                        ps = pjp.tile([128, 512], F32, tag="pj",
                                      name=f"pjps{qb}{nt}{ob}")
                        for kt2 in range(2):
                            nc.tensor.matmul(
                                ps[:, :],
                                ao_sb[:, kt2, nt * 128:(nt + 1) * 128],
                                wproj_sb[:, kt2, ob * 512:(ob + 1) * 512],
                                start=(kt2 == 0), stop=(kt2 == 1))
                        nc.vector.tensor_copy(
                            outst[:, ob * 512:(ob + 1) * 512], ps[:, :])
                    nc.sync.dma_start(
                        out=out_d[nt * 128:(nt + 1) * 128, :], in_=outst[:, :])
                return emit

            # ---- normalize tail, split so the PE bc matmul lands inside the
            # next chain (recip has time to finish on the DVE) ----

            def norm_pvs(hp, qb, pv_e, pv_o):
                """Copy the pv accumulators to SBUF: frees their PSUM banks
                quickly. Reciprocals/normalize run 1-2 chains later."""
                pvs = stpool.tile([65, 1024], F32, tag="pvs",
                                  name=f"pvs{hp}{qb}")
                nc.vector.tensor_copy(pvs[0:65, 0:512], pv_e[0:65, :])
                nc.vector.tensor_copy(pvs[0:65, 512:1024], pv_o[0:65, :])
                return pvs

            def norm_recips(e):
                rcr = smpool.tile([65, 1024], BF16, tag="rcr",
                                  name=f"rcr{e['hp']}{e['qb']}")
                nc.vector.reciprocal(rcr[64:65, 0:512], e['pvs'][64:65, 0:512])
                nc.vector.reciprocal(rcr[64:65, 512:1024],
                                     e['pvs'][64:65, 512:1024])
                return rcr

            def norm_apply(e):
                """bc matmul (PE, bf16) + DVE multiply + DMA into ao_sb."""
                hp, qb, parity = e['hp'], e['qb'], e['applied']
                pvs, rcr = e['pvs'], e['rcr']
                pi = parity * 64
                bc = pjp.tile([64, 512], F32, tag="pj", name=f"bc{hp}{qb}{parity}")
                nc.tensor.matmul(
                    bc[:, :], ones_bf[64:65, 0:64],
                    rcr[64:65, parity * 512:(parity + 1) * 512],
                    start=True, stop=True)
                aos = stpool.tile([64, 512], BF16, tag="aos",
                                  name=f"aos{hp}{qb}{parity}")
                nc.vector.tensor_mul(
                    aos[:, :], bc[:, :],
                    pvs[0:64, parity * 512:(parity + 1) * 512])
                nc.sync.dma_start(
                    out=ao_sb[pi:pi + 64, hp, qb * 512:(qb + 1) * 512],
                    in_=aos[:, :])
                e['applied'] += 1

            # ---- attention chain per head pair, with filler injection ----
            pending = []   # [{ci, hp, qb, pvs, rcr, applied}, ...]
            chain_no = [0]

            def chain(hp, qb, fillers):
                """fillers: dict slot -> list of emit closures."""
                ci = chain_no[0]
                chain_no[0] += 1
                mq, mk = hp, 2 + hp
                qT_e = qk_sb[0:64, mq, qb * 512:(qb + 1) * 512]
                qT_o = qk_sb[64:128, mq, qb * 512:(qb + 1) * 512]
                pv_e = pvp.tile([128, 512], F32, tag="pv", name=f"pve{hp}{qb}")
                pv_o = pvp.tile([128, 512], F32, tag="pv", name=f"pvo{hp}{qb}")
                for kt in range(NT):
                    sc = scp.tile([128, 1024], F32, tag="sc", name=f"sc{kt}")
                    nc.tensor.matmul(
                        sc[:, 0:512],
                        qk_sb[0:64, mk, kt * 128:(kt + 1) * 128], qT_e,
                        start=True, stop=True)
                    nc.tensor.matmul(
                        sc[:, 512:1024],
                        qk_sb[64:128, mk, kt * 128:(kt + 1) * 128], qT_o,
                        start=True, stop=True)
                    pr = prpool.tile([128, 1024], BF16, tag="probs",
                                     name=f"pr{kt}")
                    nc.scalar.activation(pr[:, :], sc[:, :], Exp, scale=SCALE)
                    # previous chain's reciprocals start on the DVE at slot 1
                    # (~6.6us, done by ~slot 7); its normalize applies at
                    # slots 8/11, so neither ever blocks the PE
                    if kt == 1:
                        for e in pending:
                            if e['ci'] == ci - 1 and e['rcr'] is None:
                                e['rcr'] = norm_recips(e)
                    if kt in (8, 11):
                        for e in pending:
                            if e['ci'] <= ci - 1 and e['applied'] < 2:
                                norm_apply(e)
                                break
                    for emit in fillers.get(kt, ()):
                        emit()
                    nc.tensor.matmul(
                        pv_e[0:65, :], v_sb[:, kt, 2 * hp, 0:HD + 1],
                        pr[:, 0:512],
                        start=(kt == 0), stop=(kt == NT - 1))
                    nc.tensor.matmul(
                        pv_o[0:65, :], v_sb[:, kt, 2 * hp + 1, 0:HD + 1],
                        pr[:, 512:1024],
                        start=(kt == 0), stop=(kt == NT - 1))
                pvs = norm_pvs(hp, qb, pv_e, pv_o)
                pending.append({'ci': ci, 'hp': hp, 'qb': qb, 'pvs': pvs,
                                'rcr': None, 'applied': 0})
                while pending and pending[0]['applied'] >= 2:
                    pending.pop(0)

            c0_fills = {s: [v_fill(3 + s)] for s in range(13)}
            c0_fills[13] = [qk_fill(0, 1)]
            chain(0, 0, c0_fills)
            chain(0, 1, {1: [qk_fill(3, 0)], 4: [qk_fill(3, 1)],
                         8: [qk_fill(3, 2)], 12: [qk_fill(0, 2)]})
            chain(0, 2, {1: [qk_fill(3, 3)], 4: [qk_fill(1, 0)],
                         8: [qk_fill(1, 1)], 12: [qk_fill(0, 3)]})
            chain(0, 3, {1: [qk_fill(1, 2)], 4: [qk_fill(1, 3)]})
            chain(1, 0, {})
            chain(1, 1, {12: [proj_fill(0, 0)], 13: [proj_fill(0, 1)],
                         14: [proj_fill(0, 2)], 15: [proj_fill(0, 3)]})
            chain(1, 2, {12: [proj_fill(1, 0)], 13: [proj_fill(1, 1)],
                         14: [proj_fill(1, 2)], 15: [proj_fill(1, 3)]})
            chain(1, 3, {12: [proj_fill(2, 0)], 13: [proj_fill(2, 1)],
                         14: [proj_fill(2, 2)], 15: [proj_fill(2, 3)]})
            # tail: only the last chain's normalize + its projection remain
            for e in pending:
                if e['rcr'] is None:
                    e['rcr'] = norm_recips(e)
            for e in pending:
                while e['applied'] < 2:
                    norm_apply(e)
            for j in range(4):
                proj_fill(3, j)()

    nc.compile()
    return nc


def _get_program():
    if "nc" not in _CACHE:
        _CACHE["nc"] = _build_program()
    return _CACHE["nc"]


def _make_in_maps(x, w_qkv, w_proj):
    import ml_dtypes
    bf16 = ml_dtypes.bfloat16
    x = np.asarray(x, dtype=np.float32)
    w_qkv = np.asarray(w_qkv, dtype=np.float32)
    w_proj = np.asarray(w_proj, dtype=np.float32)
    xT = [np.ascontiguousarray(x[b].T).astype(bf16) for b in range(B)]
    in_maps = []
    for c in range(NCORES):
        b, hg = c // 4, c % 4
        rows = slice(hg * LQK, (hg + 1) * LQK)
        qk_rows = np.r_[np.arange(hg * LQK, (hg + 1) * LQK),
                        D + np.arange(hg * LQK, (hg + 1) * LQK)]
        in_maps.append({
            "xT": xT[b],
            "wqkT": np.ascontiguousarray(w_qkv[qk_rows, :].T).astype(bf16),
            "wvT": np.ascontiguousarray(
                w_qkv[2 * D + np.arange(hg * LQK, (hg + 1) * LQK), :].T).astype(bf16),
            "wprojT": np.ascontiguousarray(w_proj[:, rows].T).astype(bf16),
        })
    return in_maps


def kernel(x, w_qkv, w_proj, b_proj, _return_results=False, _trace=False):
    from concourse import bass_utils

    nc = _get_program()
    in_maps = _make_in_maps(x, w_qkv, w_proj)
    res = bass_utils.run_bass_kernel_spmd(
        nc, in_maps, list(range(NCORES)), trace=_trace)
    partials = np.stack([res.results[c]["out"] for c in range(NCORES)])
    out = partials.reshape(B, 4, N, D).sum(axis=1, dtype=np.float32)
    out = out + np.asarray(b_proj, dtype=np.float32)[None, None, :]
    out = out.astype(np.float32)
    if _return_results:
        return out, res
    return out


# revision 38
# speedup vs baseline: 1.0291x; 1.0020x over previous
"""Multi-head attention (B=2, N=2048, D=1024, H=16, hd=64) on 8 trn2 NeuronCores.

Sharding: 8 cores = 2 (batch) x 4 (head groups of 4 heads).
Core c: batch b = c // 4, heads hg*4 .. hg*4+3 where hg = c % 4.

Per-core program (identical SPMD program, per-core data):
  inputs (DRAM):
    xT     [1024, 2048]  = x[b].T
    wqkT   [1024, 512]   = w_qkv[[q rows, k rows] of local heads].T
    wvT    [1024, 256]   = w_qkv[v rows of local heads].T
    wprojT [256, 1024]   = w_proj[:, local head cols].T
  output:
    out    [2048, 1024]  partial (row-parallel) projection output

Pipeline (v2, ACT-exp is the rail at ~146us):
  - During the x DMA (12.6us floor), accumulate k-gemm (m2) and half the
    v-gemm kt-outer so the PE works while x streams in.
  - Attention chains per head-PAIR with row-tiled score matmuls: head even
    at array rows 0-63 (tile_position (0,0)), head odd at rows 64-127
    ((64,0)) -> both score MMs run concurrently; one [128,1024] exp per
    key tile covers both heads.
  - PV per head ones-augmented (65th weight column = denominator row).
  - Chain order: hp0 for qb0..3, then hp1 for qb0..3, so the hp1 q/k gemms
    and the projections become PE filler inside the ACT-bound stream.
  - Normalize: reciprocal_approx_fast (DVE) on the denominator row, f32r
    outer-product broadcast on the PE (interleaved into the next chain),
    DVE multiply, DMA into proj layout.

Host unshard: out[b] = sum over 4 head-group partials + b_proj.
"""

import sys

if "/opt/trn_rl_repo" not in sys.path:
    sys.path.insert(0, "/opt/trn_rl_repo")

import numpy as np

B, N, D, H, HD = 2, 2048, 1024, 16, 64
NCORES = 8
HPC = 4               # heads per core
LQK = HPC * HD        # 256 local q (or k) rows
SCALE = HD ** -0.5    # 0.125

_CACHE = {}


def _build_program():
    import concourse.tile as tile
    from concourse import bacc, mybir

    F32 = mybir.dt.float32
    BF16 = mybir.dt.bfloat16
    Exp = mybir.ActivationFunctionType.Exp

    nc = bacc.Bacc("TRN2", target_bir_lowering=False, debug=False,
                   num_devices=NCORES)

    xT_d = nc.dram_tensor("xT", [D, N], BF16, kind="ExternalInput").ap()
    wqkT_d = nc.dram_tensor("wqkT", [D, 2 * LQK], BF16, kind="ExternalInput").ap()
    wvT_d = nc.dram_tensor("wvT", [D, LQK], BF16, kind="ExternalInput").ap()
    wprojT_d = nc.dram_tensor("wprojT", [LQK, D], BF16, kind="ExternalInput").ap()
    out_d = nc.dram_tensor("out", [N, D], F32, kind="ExternalOutput").ap()

    KT = D // 128        # 8 contraction tiles for qkv gemms
    NB = N // 512        # 4 seq blocks
    NT = N // 128        # 16 seq tiles

    with tile.TileContext(nc) as tc:
        with (
            nc.allow_low_precision(reason="bf16 matmul operands"),
            tc.tile_pool(name="const", bufs=1) as cpool,
            tc.tile_pool(name="w", bufs=1) as wpool,
            tc.tile_pool(name="x", bufs=1) as xpool,
            tc.tile_pool(name="qk", bufs=1) as qkpool,
            tc.tile_pool(name="vaug", bufs=1) as vapool,
            tc.tile_pool(name="ao", bufs=1) as aopool,
            tc.tile_pool(name="probs", bufs=3) as prpool,
            tc.tile_pool(name="small", bufs=4) as smpool,
            tc.tile_pool(name="stage", bufs=3) as stpool,
            tc.tile_pool(name="scp", bufs=2, space="PSUM") as scp,   # 2x[128,1024]
            tc.tile_pool(name="pvp", bufs=2, space="PSUM") as pvp,   # 2x[128,512]
            tc.tile_pool(name="pjp", bufs=2, space="PSUM") as pjp,   # 2x[128,512]
        ):
            ones_f32 = cpool.tile([128, 128], F32)
            nc.vector.memset(ones_f32[:, :], 1.0)
            ones_bf = cpool.tile([65, 128], BF16)
            nc.vector.tensor_copy(ones_bf[:, :], ones_f32[0:65, :])

            # ---- input DMAs (kt-chunked so compute starts early) ----
            x_sb = xpool.tile([128, KT, N], BF16)
            wqk_sb = wpool.tile([128, KT, 2 * LQK], BF16)
            wv_sb = wpool.tile([128, KT, LQK], BF16)
            xT_r = xT_d.rearrange("(kt p) n -> p kt n", p=128)
            wqkT_r = wqkT_d.rearrange("(kt p) m -> p kt m", p=128)
            wvT_r = wvT_d.rearrange("(kt p) m -> p kt m", p=128)
            # One queue processes transfers in order, so: whole wqk first
            # (kq gemm weights), first x chunk, wv, then remaining x chunks
            # kt-granular so the kt-outer gemm tracks arrivals; wproj last.
            nc.sync.dma_start(out=wqk_sb[:, :, :], in_=wqkT_r[:, :, :])
            nc.sync.dma_start(out=x_sb[:, 0, :], in_=xT_r[:, 0, :])
            nc.sync.dma_start(out=wv_sb[:, :, :], in_=wvT_r[:, :, :])
            for kt in range(1, KT):
                nc.sync.dma_start(out=x_sb[:, kt, :], in_=xT_r[:, kt, :])
            wproj_sb = wpool.tile([128, 2, D], BF16)
            nc.sync.dma_start(
                out=wproj_sb[:, :, :],
                in_=wprojT_d.rearrange("(kt p) o -> p kt o", p=128))

            # qk_sb m-tile layout: m=0: q heads 0,1 / m=1: q heads 2,3
            #                      m=2: k heads 0,1 / m=3: k heads 2,3
            qk_sb = qkpool.tile([128, 4, N], BF16)
            v_sb = vapool.tile([128, NT, HPC, HD + 1], BF16)
            # proj lhsT: kt2=hp, partitions: even head 0:64, odd head 64:128
            ao_sb = aopool.tile([128, 2, N], BF16)

            WCOL = (0, 128, 256, 384)  # wqkT col base per m-tile

            def v_copy(ps, st0, cnt):
                """Copy cnt [128, LQK] psum v-slices (st0..) into v_sb."""
                for j in range(cnt):
                    nc.vector.tensor_copy(
                        v_sb[:, st0 + j, :, 0:HD],
                        ps[:, j * LQK:(j + 1) * LQK].rearrange(
                            "p (h d) -> p h d", h=HPC))
                    nc.vector.tensor_copy(
                        v_sb[:, st0 + j, :, HD:HD + 1],
                        ones_f32[:, 0:HPC].rearrange("p (h c) -> p h c", c=1))

            # ---- gemm phase overlapped with the x DMA (kt-outer) ----
            # NOTE: interleaved accumulation groups must not share a PSUM
            # bank (start=True clears bank-wide state), so each concurrent
            # accumulation gets its own bank: m2 nb0-3 (2 banks each of the
            # two scp tiles), v st0/st1/st2 (one bank each), m0 nb0.
            m2ps_a = scp.tile([128, 1024], F32, tag="sc")
            m2ps_b = scp.tile([128, 1024], F32, tag="sc")
            vps0 = pvp.tile([128, 512], F32, tag="pv")
            vps1 = pvp.tile([128, 512], F32, tag="pv")
            vps2 = pjp.tile([128, 512], F32, tag="pj")
            m0ps = pjp.tile([128, 512], F32, tag="pj")
            for kt in range(KT):
                st_, sp_ = (kt == 0), (kt == KT - 1)
                for nb in range(NB):
                    ps = m2ps_a if nb < 2 else m2ps_b
                    nc.tensor.matmul(
                        ps[:, (nb % 2) * 512:(nb % 2 + 1) * 512],
                        wqk_sb[:, kt, WCOL[2]:WCOL[2] + 128],
                        x_sb[:, kt, nb * 512:(nb + 1) * 512],
                        start=st_, stop=sp_)
                nc.tensor.matmul(
                    m0ps[:, :],
                    wqk_sb[:, kt, WCOL[0]:WCOL[0] + 128],
                    x_sb[:, kt, 0:512],
                    start=st_, stop=sp_)
                for st in range(3):
                    ps = (vps0, vps1, vps2)[st]
                    nc.tensor.matmul(
                        ps[:, 0:LQK],
                        x_sb[:, kt, st * 128:(st + 1) * 128],
                        wv_sb[:, kt, :],
                        start=st_, stop=sp_)
            # copies ordered so C0's first tiles unblock first
            nc.vector.tensor_copy(qk_sb[:, 0, 0:512], m0ps[:, :])
            nc.vector.tensor_copy(qk_sb[:, 2, 0:512], m2ps_a[:, 0:512])
            v_copy(vps0, 0, 1)
            nc.vector.tensor_copy(qk_sb[:, 2, 512:1024], m2ps_a[:, 512:1024])
            v_copy(vps1, 1, 1)
            v_copy(vps2, 2, 1)
            nc.vector.tensor_copy(qk_sb[:, 2, 1024:1536], m2ps_b[:, 0:512])
            nc.vector.tensor_copy(qk_sb[:, 2, 1536:2048], m2ps_b[:, 512:1024])

            # ---- filler generators (kt-inner gemm pieces fed into chains) --

            def qk_fill(m, nb):
                def emit():
                    ps = pjp.tile([128, 512], F32, tag="pj", name=f"qkf{m}{nb}")
                    for kt in range(KT):
                        nc.tensor.matmul(
                            ps[:, :],
                            wqk_sb[:, kt, WCOL[m]:WCOL[m] + 128],
                            x_sb[:, kt, nb * 512:(nb + 1) * 512],
                            start=(kt == 0), stop=(kt == KT - 1))
                    nc.vector.tensor_copy(
                        qk_sb[:, m, nb * 512:(nb + 1) * 512], ps[:, :])
                return emit

            def v_fill(st0):
                def emit():
                    ps = pjp.tile([128, 512], F32, tag="pj", name=f"vf{st0}")
                    for kt in range(KT):
                        nc.tensor.matmul(
                            ps[:, 0:LQK],
                            x_sb[:, kt, st0 * 128:(st0 + 1) * 128],
                            wv_sb[:, kt, :],
                            start=(kt == 0), stop=(kt == KT - 1))
                    v_copy(ps, st0, 1)
                return emit

            def proj_fill(qb, j):
                nt = qb * 4 + j

                def emit():
                    outst = stpool.tile([128, 1024], F32, tag="outst",
                                        name=f"outst{qb}{nt}")
                    for ob in range(2):
                        ps = pjp.tile([128, 512], F32, tag="pj",
                                      name=f"pjps{qb}{nt}{ob}")
                        for kt2 in range(2):
                            nc.tensor.matmul(
                                ps[:, :],
                                ao_sb[:, kt2, nt * 128:(nt + 1) * 128],
                                wproj_sb[:, kt2, ob * 512:(ob + 1) * 512],
                                start=(kt2 == 0), stop=(kt2 == 1))
                        nc.vector.tensor_copy(
                            outst[:, ob * 512:(ob + 1) * 512], ps[:, :])
                    nc.sync.dma_start(
                        out=out_d[nt * 128:(nt + 1) * 128, :], in_=outst[:, :])
                return emit

            # ---- normalize tail, split so the PE bc matmul lands inside the
            # next chain (recip has time to finish on the DVE) ----

            def norm_pvs(hp, qb, pv_e, pv_o):
                """Copy the pv accumulators to SBUF: frees their PSUM banks
                quickly. Reciprocals/normalize run 1-2 chains later."""
                pvs = stpool.tile([65, 1024], F32, tag="pvs",
                                  name=f"pvs{hp}{qb}")
                nc.vector.tensor_copy(pvs[0:65, 0:512], pv_e[0:65, :])
                nc.vector.tensor_copy(pvs[0:65, 512:1024], pv_o[0:65, :])
                return pvs

            def norm_recips(e):
                rcr = smpool.tile([65, 1024], BF16, tag="rcr",
                                  name=f"rcr{e['hp']}{e['qb']}")
                nc.vector.reciprocal(rcr[64:65, 0:512], e['pvs'][64:65, 0:512])
                nc.vector.reciprocal(rcr[64:65, 512:1024],
                                     e['pvs'][64:65, 512:1024])
                return rcr

            def norm_apply(e):
                """bc matmul (PE, bf16) + DVE multiply + DMA into ao_sb."""
                hp, qb, parity = e['hp'], e['qb'], e['applied']
                pvs, rcr = e['pvs'], e['rcr']
                pi = parity * 64
                bc = pjp.tile([64, 512], F32, tag="pj", name=f"bc{hp}{qb}{parity}")
                nc.tensor.matmul(
                    bc[:, :], ones_bf[64:65, 0:64],
                    rcr[64:65, parity * 512:(parity + 1) * 512],
                    start=True, stop=True)
                aos = stpool.tile([64, 512], BF16, tag="aos",
                                  name=f"aos{hp}{qb}{parity}")
                nc.vector.tensor_mul(
                    aos[:, :], bc[:, :],
                    pvs[0:64, parity * 512:(parity + 1) * 512])
                nc.sync.dma_start(
                    out=ao_sb[pi:pi + 64, hp, qb * 512:(qb + 1) * 512],
                    in_=aos[:, :])
                e['applied'] += 1

            # ---- attention chain per head pair, with filler injection ----
            pending = []   # [{ci, hp, qb, pvs, rcr, applied}, ...]
            chain_no = [0]

            def chain(hp, qb, fillers):
                """fillers: dict slot -> list of emit closures."""
                ci = chain_no[0]
                chain_no[0] += 1
                mq, mk = hp, 2 + hp
                qT_e = qk_sb[0:64, mq, qb * 512:(qb + 1) * 512]
                qT_o = qk_sb[64:128, mq, qb * 512:(qb + 1) * 512]
                pv_e = pvp.tile([128, 512], F32, tag="pv", name=f"pve{hp}{qb}")
                pv_o = pvp.tile([128, 512], F32, tag="pv", name=f"pvo{hp}{qb}")
                for kt in range(NT):
                    sc = scp.tile([128, 1024], F32, tag="sc", name=f"sc{kt}")
                    nc.tensor.matmul(
                        sc[:, 0:512],
                        qk_sb[0:64, mk, kt * 128:(kt + 1) * 128], qT_e,
                        start=True, stop=True)
                    nc.tensor.matmul(
                        sc[:, 512:1024],
                        qk_sb[64:128, mk, kt * 128:(kt + 1) * 128], qT_o,
                        start=True, stop=True)
                    pr = prpool.tile([128, 1024], BF16, tag="probs",
                                     name=f"pr{kt}")
                    nc.scalar.activation(pr[:, :], sc[:, :], Exp, scale=SCALE)
                    # previous chain's reciprocals start on the DVE at slot 1
                    # (~6.6us, done by ~slot 7); its normalize applies at
                    # slots 8/11, so neither ever blocks the PE
                    if kt == 1:
                        for e in pending:
                            if e['ci'] == ci - 1 and e['rcr'] is None:
                                e['rcr'] = norm_recips(e)
                    if kt in (10, 12):
                        for e in pending:
                            if e['ci'] <= ci - 1 and e['applied'] < 2:
                                norm_apply(e)
                                break
                    for emit in fillers.get(kt, ()):
                        emit()
                    nc.tensor.matmul(
                        pv_e[0:65, :], v_sb[:, kt, 2 * hp, 0:HD + 1],
                        pr[:, 0:512],
                        start=(kt == 0), stop=(kt == NT - 1))
                    nc.tensor.matmul(
                        pv_o[0:65, :], v_sb[:, kt, 2 * hp + 1, 0:HD + 1],
                        pr[:, 512:1024],
                        start=(kt == 0), stop=(kt == NT - 1))
                pvs = norm_pvs(hp, qb, pv_e, pv_o)
                pending.append({'ci': ci, 'hp': hp, 'qb': qb, 'pvs': pvs,
                                'rcr': None, 'applied': 0})
                while pending and pending[0]['applied'] >= 2:
                    pending.pop(0)

            c0_fills = {s: [v_fill(3 + s)] for s in range(13)}
            c0_fills[13] = [qk_fill(0, 1)]
            chain(0, 0, c0_fills)
            chain(0, 1, {1: [qk_fill(3, 0)], 4: [qk_fill(3, 1)],
                         8: [qk_fill(3, 2)], 12: [qk_fill(0, 2)]})
            chain(0, 2, {1: [qk_fill(3, 3)], 4: [qk_fill(1, 0)],
                         8: [qk_fill(1, 1)], 12: [qk_fill(0, 3)]})
            chain(0, 3, {1: [qk_fill(1, 2)], 4: [qk_fill(1, 3)]})
            chain(1, 0, {})
            chain(1, 1, {13: [proj_fill(0, 0)], 14: [proj_fill(0, 1)],
                         15: [proj_fill(0, 2)]})
            chain(1, 2, {0: [proj_fill(0, 3)], 13: [proj_fill(1, 0)],
                         14: [proj_fill(1, 1)], 15: [proj_fill(1, 2)]})
            chain(1, 3, {0: [proj_fill(1, 3)], 13: [proj_fill(2, 0)],
                         14: [proj_fill(2, 1)], 15: [proj_fill(2, 2)]})
            # tail: spillover proj item, last chain's normalize, final proj
            for e in pending:
                if e['rcr'] is None:
                    e['rcr'] = norm_recips(e)
            proj_fill(2, 3)()
            for e in pending:
                while e['applied'] < 2:
                    norm_apply(e)
            for j in range(4):
                proj_fill(3, j)()

    nc.compile()
    return nc


def _get_program():
    if "nc" not in _CACHE:
        _CACHE["nc"] = _build_program()
    return _CACHE["nc"]


def _make_in_maps(x, w_qkv, w_proj):
    import ml_dtypes
    bf16 = ml_dtypes.bfloat16
    x = np.asarray(x, dtype=np.float32)
    w_qkv = np.asarray(w_qkv, dtype=np.float32)
    w_proj = np.asarray(w_proj, dtype=np.float32)
    xT = [np.ascontiguousarray(x[b].T).astype(bf16) for b in range(B)]
    in_maps = []
    for c in range(NCORES):
        b, hg = c // 4, c % 4
        rows = slice(hg * LQK, (hg + 1) * LQK)
        qk_rows = np.r_[np.arange(hg * LQK, (hg + 1) * LQK),
                        D + np.arange(hg * LQK, (hg + 1) * LQK)]
        in_maps.append({
            "xT": xT[b],
            "wqkT": np.ascontiguousarray(w_qkv[qk_rows, :].T).astype(bf16),
            "wvT": np.ascontiguousarray(
                w_qkv[2 * D + np.arange(hg * LQK, (hg + 1) * LQK), :].T).astype(bf16),
            "wprojT": np.ascontiguousarray(w_proj[:, rows].T).astype(bf16),
        })
    return in_maps


def kernel(x, w_qkv, w_proj, b_proj, _return_results=False, _trace=False):
    from concourse import bass_utils

    nc = _get_program()
    in_maps = _make_in_maps(x, w_qkv, w_proj)
    res = bass_utils.run_bass_kernel_spmd(
        nc, in_maps, list(range(NCORES)), trace=_trace)
    partials = np.stack([res.results[c]["out"] for c in range(NCORES)])
    out = partials.reshape(B, 4, N, D).sum(axis=1, dtype=np.float32)
    out = out + np.asarray(b_proj, dtype=np.float32)[None, None, :]
    out = out.astype(np.float32)
    if _return_results:
        return out, res
    return out


# revision 39
# speedup vs baseline: 1.0555x; 1.0257x over previous
"""Multi-head attention (B=2, N=2048, D=1024, H=16, hd=64) on 8 trn2 NeuronCores.

Sharding: 8 cores = 2 (batch) x 4 (head groups of 4 heads).
Core c: batch b = c // 4, heads hg*4 .. hg*4+3 where hg = c % 4.

Per-core program (identical SPMD program, per-core data):
  inputs (DRAM):
    xT     [1024, 2048]  = x[b].T
    wqkT   [1024, 512]   = w_qkv[[q rows, k rows] of local heads].T
    wvT    [1024, 256]   = w_qkv[v rows of local heads].T
    wprojT [256, 1024]   = w_proj[:, local head cols].T
  output:
    out    [2048, 1024]  partial (row-parallel) projection output

Pipeline (v2, ACT-exp is the rail at ~146us):
  - During the x DMA (12.6us floor), accumulate k-gemm (m2) and half the
    v-gemm kt-outer so the PE works while x streams in.
  - Attention chains per head-PAIR with row-tiled score matmuls: head even
    at array rows 0-63 (tile_position (0,0)), head odd at rows 64-127
    ((64,0)) -> both score MMs run concurrently; one [128,1024] exp per
    key tile covers both heads.
  - PV per head ones-augmented (65th weight column = denominator row).
  - Chain order: hp0 for qb0..3, then hp1 for qb0..3, so the hp1 q/k gemms
    and the projections become PE filler inside the ACT-bound stream.
  - Normalize: reciprocal_approx_fast (DVE) on the denominator row, f32r
    outer-product broadcast on the PE (interleaved into the next chain),
    DVE multiply, DMA into proj layout.

Host unshard: out[b] = sum over 4 head-group partials + b_proj.
"""

import sys

if "/opt/trn_rl_repo" not in sys.path:
    sys.path.insert(0, "/opt/trn_rl_repo")

import numpy as np

B, N, D, H, HD = 2, 2048, 1024, 16, 64
NCORES = 8
HPC = 4               # heads per core
LQK = HPC * HD        # 256 local q (or k) rows
SCALE = HD ** -0.5    # 0.125

_CACHE = {}


def _build_program():
    import concourse.tile as tile
    from concourse import bacc, mybir

    F32 = mybir.dt.float32
    BF16 = mybir.dt.bfloat16
    Exp = mybir.ActivationFunctionType.Exp

    nc = bacc.Bacc("TRN2", target_bir_lowering=False, debug=False,
                   num_devices=NCORES)

    xT_d = nc.dram_tensor("xT", [D, N], BF16, kind="ExternalInput").ap()
    wqkT_d = nc.dram_tensor("wqkT", [D, 2 * LQK], BF16, kind="ExternalInput").ap()
    wvT_d = nc.dram_tensor("wvT", [D, LQK], BF16, kind="ExternalInput").ap()
    wprojT_d = nc.dram_tensor("wprojT", [LQK, D], BF16, kind="ExternalInput").ap()
    out_d = nc.dram_tensor("out", [N, D], F32, kind="ExternalOutput").ap()

    KT = D // 128        # 8 contraction tiles for qkv gemms
    NB = N // 512        # 4 seq blocks
    NT = N // 128        # 16 seq tiles

    with tile.TileContext(nc) as tc:
        with (
            nc.allow_low_precision(reason="bf16 matmul operands"),
            tc.tile_pool(name="const", bufs=1) as cpool,
            tc.tile_pool(name="w", bufs=1) as wpool,
            tc.tile_pool(name="x", bufs=1) as xpool,
            tc.tile_pool(name="qk", bufs=1) as qkpool,
            tc.tile_pool(name="vaug", bufs=1) as vapool,
            tc.tile_pool(name="ao", bufs=1) as aopool,
            tc.tile_pool(name="probs", bufs=3) as prpool,
            tc.tile_pool(name="small", bufs=4) as smpool,
            tc.tile_pool(name="stage", bufs=3) as stpool,
            tc.tile_pool(name="scp", bufs=2, space="PSUM") as scp,   # 2x[128,1024]
            tc.tile_pool(name="pvp", bufs=2, space="PSUM") as pvp,   # 2x[128,512]
            tc.tile_pool(name="pjp", bufs=2, space="PSUM") as pjp,   # 2x[128,512]
        ):
            ones_f32 = cpool.tile([128, 128], F32)
            nc.vector.memset(ones_f32[:, :], 1.0)
            ones_bf = cpool.tile([65, 128], BF16)
            nc.vector.tensor_copy(ones_bf[:, :], ones_f32[0:65, :])

            # ---- input DMAs (kt-chunked so compute starts early) ----
            x_sb = xpool.tile([128, KT, N], BF16)
            wqk_sb = wpool.tile([128, KT, 2 * LQK], BF16)
            wv_sb = wpool.tile([128, KT, LQK], BF16)
            xT_r = xT_d.rearrange("(kt p) n -> p kt n", p=128)
            wqkT_r = wqkT_d.rearrange("(kt p) m -> p kt m", p=128)
            wvT_r = wvT_d.rearrange("(kt p) m -> p kt m", p=128)
            # One queue processes transfers in order, so: whole wqk first
            # (kq gemm weights), first x chunk, wv, then remaining x chunks
            # kt-granular so the kt-outer gemm tracks arrivals; wproj last.
            nc.sync.dma_start(out=wqk_sb[:, :, :], in_=wqkT_r[:, :, :])
            nc.sync.dma_start(out=x_sb[:, 0, :], in_=xT_r[:, 0, :])
            nc.sync.dma_start(out=wv_sb[:, :, :], in_=wvT_r[:, :, :])
            for kt in range(1, KT):
                nc.sync.dma_start(out=x_sb[:, kt, :], in_=xT_r[:, kt, :])
            wproj_sb = wpool.tile([128, 2, D], BF16)
            nc.sync.dma_start(
                out=wproj_sb[:, :, :],
                in_=wprojT_d.rearrange("(kt p) o -> p kt o", p=128))

            # qk_sb m-tile layout: m=0: q heads 0,1 / m=1: q heads 2,3
            #                      m=2: k heads 0,1 / m=3: k heads 2,3
            qk_sb = qkpool.tile([128, 4, N], BF16)
            v_sb = vapool.tile([128, NT, HPC, HD + 1], BF16)
            # proj lhsT: kt2=hp, partitions: even head 0:64, odd head 64:128
            ao_sb = aopool.tile([128, 2, N], BF16)

            WCOL = (0, 128, 256, 384)  # wqkT col base per m-tile

            def v_copy(ps, st0, cnt):
                """Copy cnt [128, LQK] psum v-slices (st0..) into v_sb."""
                for j in range(cnt):
                    nc.vector.tensor_copy(
                        v_sb[:, st0 + j, :, 0:HD],
                        ps[:, j * LQK:(j + 1) * LQK].rearrange(
                            "p (h d) -> p h d", h=HPC))
                    nc.vector.tensor_copy(
                        v_sb[:, st0 + j, :, HD:HD + 1],
                        ones_f32[:, 0:HPC].rearrange("p (h c) -> p h c", c=1))

            # ---- gemm phase overlapped with the x DMA (kt-outer) ----
            # NOTE: interleaved accumulation groups must not share a PSUM
            # bank (start=True clears bank-wide state), so each concurrent
            # accumulation gets its own bank: m2 nb0-3 (2 banks each of the
            # two scp tiles), v st0/st1/st2 (one bank each), m0 nb0.
            m2ps_a = scp.tile([128, 1024], F32, tag="sc")
            m2ps_b = scp.tile([128, 1024], F32, tag="sc")
            vps0 = pvp.tile([128, 512], F32, tag="pv")
            vps1 = pvp.tile([128, 512], F32, tag="pv")
            vps2 = pjp.tile([128, 512], F32, tag="pj")
            m0ps = pjp.tile([128, 512], F32, tag="pj")
            for kt in range(KT):
                st_, sp_ = (kt == 0), (kt == KT - 1)
                for nb in range(NB):
                    ps = m2ps_a if nb < 2 else m2ps_b
                    nc.tensor.matmul(
                        ps[:, (nb % 2) * 512:(nb % 2 + 1) * 512],
                        wqk_sb[:, kt, WCOL[2]:WCOL[2] + 128],
                        x_sb[:, kt, nb * 512:(nb + 1) * 512],
                        start=st_, stop=sp_)
                nc.tensor.matmul(
                    m0ps[:, :],
                    wqk_sb[:, kt, WCOL[0]:WCOL[0] + 128],
                    x_sb[:, kt, 0:512],
                    start=st_, stop=sp_)
                for st in range(3):
                    ps = (vps0, vps1, vps2)[st]
                    nc.tensor.matmul(
                        ps[:, 0:LQK],
                        x_sb[:, kt, st * 128:(st + 1) * 128],
                        wv_sb[:, kt, :],
                        start=st_, stop=sp_)
            # copies ordered so C0's first tiles unblock first
            nc.vector.tensor_copy(qk_sb[:, 0, 0:512], m0ps[:, :])
            nc.vector.tensor_copy(qk_sb[:, 2, 0:512], m2ps_a[:, 0:512])
            v_copy(vps0, 0, 1)
            nc.vector.tensor_copy(qk_sb[:, 2, 512:1024], m2ps_a[:, 512:1024])
            v_copy(vps1, 1, 1)
            v_copy(vps2, 2, 1)
            nc.vector.tensor_copy(qk_sb[:, 2, 1024:1536], m2ps_b[:, 0:512])
            nc.vector.tensor_copy(qk_sb[:, 2, 1536:2048], m2ps_b[:, 512:1024])

            # ---- filler generators (kt-inner gemm pieces fed into chains) --

            def qk_fill(m, nb):
                def emit():
                    ps = pjp.tile([128, 512], F32, tag="pj", name=f"qkf{m}{nb}")
                    for kt in range(KT):
                        nc.tensor.matmul(
                            ps[:, :],
                            wqk_sb[:, kt, WCOL[m]:WCOL[m] + 128],
                            x_sb[:, kt, nb * 512:(nb + 1) * 512],
                            start=(kt == 0), stop=(kt == KT - 1))
                    nc.vector.tensor_copy(
                        qk_sb[:, m, nb * 512:(nb + 1) * 512], ps[:, :])
                return emit

            def v_fill(st0):
                def emit():
                    ps = pjp.tile([128, 512], F32, tag="pj", name=f"vf{st0}")
                    for kt in range(KT):
                        nc.tensor.matmul(
                            ps[:, 0:LQK],
                            x_sb[:, kt, st0 * 128:(st0 + 1) * 128],
                            wv_sb[:, kt, :],
                            start=(kt == 0), stop=(kt == KT - 1))
                    v_copy(ps, st0, 1)
                return emit

            def proj_fill(qb, j):
                nt = qb * 4 + j

                def emit():
                    outst = stpool.tile([128, 1024], F32, tag="outst",
                                        name=f"outst{qb}{nt}")
                    for ob in range(2):
                        ps = pjp.tile([128, 512], F32, tag="pj",
                                      name=f"pjps{qb}{nt}{ob}")
                        for kt2 in range(2):
                            nc.tensor.matmul(
                                ps[:, :],
                                ao_sb[:, kt2, nt * 128:(nt + 1) * 128],
                                wproj_sb[:, kt2, ob * 512:(ob + 1) * 512],
                                start=(kt2 == 0), stop=(kt2 == 1))
                        nc.vector.tensor_copy(
                            outst[:, ob * 512:(ob + 1) * 512], ps[:, :])
                    nc.sync.dma_start(
                        out=out_d[nt * 128:(nt + 1) * 128, :], in_=outst[:, :])
                return emit

            # ---- normalize tail, split so the PE bc matmul lands inside the
            # next chain (recip has time to finish on the DVE) ----

            def norm_pvs(hp, qb, pv_e, pv_o):
                """Copy the pv accumulators to SBUF: frees their PSUM banks
                quickly. Reciprocals/normalize run 1-2 chains later."""
                pvs = stpool.tile([65, 1024], F32, tag="pvs",
                                  name=f"pvs{hp}{qb}")
                nc.vector.tensor_copy(pvs[0:65, 0:512], pv_e[0:65, :])
                nc.vector.tensor_copy(pvs[0:65, 512:1024], pv_o[0:65, :])
                return pvs

            def norm_recips(e):
                rcr = smpool.tile([65, 1024], BF16, tag="rcr",
                                  name=f"rcr{e['hp']}{e['qb']}")
                nc.vector.reciprocal(rcr[64:65, 0:512], e['pvs'][64:65, 0:512])
                nc.vector.reciprocal(rcr[64:65, 512:1024],
                                     e['pvs'][64:65, 512:1024])
                return rcr

            def norm_apply(e):
                """bc matmul (PE, bf16) + DVE multiply + DMA into ao_sb."""
                hp, qb, parity = e['hp'], e['qb'], e['applied']
                pvs, rcr = e['pvs'], e['rcr']
                pi = parity * 64
                bc = pjp.tile([64, 512], F32, tag="pj", name=f"bc{hp}{qb}{parity}")
                nc.tensor.matmul(
                    bc[:, :], ones_bf[64:65, 0:64],
                    rcr[64:65, parity * 512:(parity + 1) * 512],
                    start=True, stop=True)
                aos = stpool.tile([64, 512], BF16, tag="aos",
                                  name=f"aos{hp}{qb}{parity}")
                nc.vector.tensor_mul(
                    aos[:, :], bc[:, :],
                    pvs[0:64, parity * 512:(parity + 1) * 512])
                nc.sync.dma_start(
                    out=ao_sb[pi:pi + 64, hp, qb * 512:(qb + 1) * 512],
                    in_=aos[:, :])
                e['applied'] += 1

            # ---- attention chain per head pair, with filler injection ----
            pending = []   # [{ci, hp, qb, pvs, rcr, applied}, ...]
            chain_no = [0]

            def chain(hp, qb, fillers):
                """fillers: dict slot -> list of emit closures."""
                ci = chain_no[0]
                chain_no[0] += 1
                mq, mk = hp, 2 + hp
                qT_e = qk_sb[0:64, mq, qb * 512:(qb + 1) * 512]
                qT_o = qk_sb[64:128, mq, qb * 512:(qb + 1) * 512]
                pv_e = pvp.tile([128, 512], F32, tag="pv", name=f"pve{hp}{qb}")
                pv_o = pvp.tile([128, 512], F32, tag="pv", name=f"pvo{hp}{qb}")
                for kt in range(NT):
                    sc = scp.tile([128, 1024], F32, tag="sc", name=f"sc{kt}")
                    nc.tensor.matmul(
                        sc[:, 0:512],
                        qk_sb[0:64, mk, kt * 128:(kt + 1) * 128], qT_e,
                        start=True, stop=True)
                    nc.tensor.matmul(
                        sc[:, 512:1024],
                        qk_sb[64:128, mk, kt * 128:(kt + 1) * 128], qT_o,
                        start=True, stop=True)
                    pr = prpool.tile([128, 1024], BF16, tag="probs",
                                     name=f"pr{kt}")
                    nc.scalar.activation(pr[:, :], sc[:, :], Exp, scale=SCALE)
                    # previous chain's reciprocals start on the DVE at slot 1
                    # (~6.6us, done by ~slot 7); its normalize applies at
                    # slots 8/11, so neither ever blocks the PE
                    if kt == 1:
                        for e in pending:
                            if e['ci'] == ci - 1 and e['rcr'] is None:
                                e['rcr'] = norm_recips(e)
                    if kt in (10, 12):
                        for e in pending:
                            if e['ci'] <= ci - 1 and e['applied'] < 2:
                                norm_apply(e)
                                break
                    for emit in fillers.get(kt, ()):
                        emit()
                    nc.tensor.matmul(
                        pv_e[0:65, :], v_sb[:, kt, 2 * hp, 0:HD + 1],
                        pr[:, 0:512],
                        start=(kt == 0), stop=(kt == NT - 1))
                    nc.tensor.matmul(
                        pv_o[0:65, :], v_sb[:, kt, 2 * hp + 1, 0:HD + 1],
                        pr[:, 512:1024],
                        start=(kt == 0), stop=(kt == NT - 1))
                pvs = norm_pvs(hp, qb, pv_e, pv_o)
                pending.append({'ci': ci, 'hp': hp, 'qb': qb, 'pvs': pvs,
                                'rcr': None, 'applied': 0})
                while pending and pending[0]['applied'] >= 2:
                    pending.pop(0)

            c0_fills = {s: [v_fill(3 + s)] for s in range(13)}
            c0_fills[13] = [qk_fill(0, 1)]
            chain(0, 0, c0_fills)
            chain(0, 1, {1: [qk_fill(3, 0)], 5: [qk_fill(3, 1)],
                         12: [qk_fill(0, 2)]})
            chain(0, 2, {1: [qk_fill(3, 2)], 5: [qk_fill(1, 0)],
                         12: [qk_fill(0, 3)]})
            chain(0, 3, {1: [qk_fill(3, 3)], 5: [qk_fill(1, 1)]})
            chain(1, 0, {1: [qk_fill(1, 2)]})
            chain(1, 1, {1: [qk_fill(1, 3)],
                         13: [proj_fill(0, 0)], 14: [proj_fill(0, 1)],
                         15: [proj_fill(0, 2)]})
            chain(1, 2, {0: [proj_fill(0, 3)], 13: [proj_fill(1, 0)],
                         14: [proj_fill(1, 1)], 15: [proj_fill(1, 2)]})
            chain(1, 3, {0: [proj_fill(1, 3)], 13: [proj_fill(2, 0)],
                         14: [proj_fill(2, 1)], 15: [proj_fill(2, 2)]})
            # tail: spillover proj item, last chain's normalize, final proj
            for e in pending:
                if e['rcr'] is None:
                    e['rcr'] = norm_recips(e)
            proj_fill(2, 3)()
            for e in pending:
                while e['applied'] < 2:
                    norm_apply(e)
            for j in range(4):
                proj_fill(3, j)()

    nc.compile()
    return nc


def _get_program():
    if "nc" not in _CACHE:
        _CACHE["nc"] = _build_program()
    return _CACHE["nc"]


def _make_in_maps(x, w_qkv, w_proj):
    import ml_dtypes
    bf16 = ml_dtypes.bfloat16
    x = np.asarray(x, dtype=np.float32)
    w_qkv = np.asarray(w_qkv, dtype=np.float32)
    w_proj = np.asarray(w_proj, dtype=np.float32)
    xT = [np.ascontiguousarray(x[b].T).astype(bf16) for b in range(B)]
    in_maps = []
    for c in range(NCORES):
        b, hg = c // 4, c % 4
        rows = slice(hg * LQK, (hg + 1) * LQK)
        qk_rows = np.r_[np.arange(hg * LQK, (hg + 1) * LQK),
                        D + np.arange(hg * LQK, (hg + 1) * LQK)]
        in_maps.append({
            "xT": xT[b],
            "wqkT": np.ascontiguousarray(w_qkv[qk_rows, :].T).astype(bf16),
            "wvT": np.ascontiguousarray(
                w_qkv[2 * D + np.arange(hg * LQK, (hg + 1) * LQK), :].T).astype(bf16),
            "wprojT": np.ascontiguousarray(w_proj[:, rows].T).astype(bf16),
        })
    return in_maps


def kernel(x, w_qkv, w_proj, b_proj, _return_results=False, _trace=False):
    from concourse import bass_utils

    nc = _get_program()
    in_maps = _make_in_maps(x, w_qkv, w_proj)
    res = bass_utils.run_bass_kernel_spmd(
        nc, in_maps, list(range(NCORES)), trace=_trace)
    partials = np.stack([res.results[c]["out"] for c in range(NCORES)])
    out = partials.reshape(B, 4, N, D).sum(axis=1, dtype=np.float32)
    out = out + np.asarray(b_proj, dtype=np.float32)[None, None, :]
    out = out.astype(np.float32)
    if _return_results:
        return out, res
    return out
